# revision 22
# baseline (speedup 1.0000x reference)
"""Trainium2 Bass kernel for Swin-style windowed attention.

Problem: x[64,196,768] -> qkv proj -> 12-head attention with relative
position bias -> out proj.  Sharded data-parallel over batch: 8 batch
items per NeuronCore across 8 cores.  All matmuls bf16 with fp32 PSUM
accumulation (measured rel err ~4e-3 vs the fp32 reference).

Per-core design (8 batch items, ~210 us on HW):
 - QKV projection: q,k feature-major ([feat, tok] so each head's 64-dim
   slice sits on partitions), v token-major ([tok, head-pair, 128]).
   x/q/k are tiled per 392-token chunk; inputs arrive via per-partition-
   contiguous repacked layouts (big DMA packets), ordered by first use.
 - Attention in the S^T layout, head pairs processed together:
   S^T[j,i] = sum_d k[d,j] q[d,i] (K=64, the pair row-packed into the
   128-deep PE array).  The pair's two S^T tiles live in ONE two-bank
   psum tile [jlen, 2, 512] (one bank per head; two accumulation chains
   in a single bank crash the exec unit), so a single strided ACT exp
   covers both heads.  Softmax runs along partitions (no max-subtract:
   logits are O(1) by construction).
 - Relative position bias applied multiplicatively after exp:
   pt = exp(S^T) * exp(rpb^T), with exp(rpb^T) precomputed on host; the
   multiply runs on the otherwise-idle GpSimd engine for the 128-row
   chunk and on DVE for the 68-row chunk.
 - PV: ONE [128, 2*196] matmul per ktok chunk with lhsT = [v_h0|v_h1]
   and rhs = [P_h0^T P_h1^T]: the diagonal 64x196 blocks are both
   heads' O^T, off-diagonal blocks are discarded -- same streamed
   columns as two matmuls, half the weight loads.  Softmax sums come
   from ones[jlen,64] matmuls (replicating each head's sums across its
   64 rows, col-packed); one fast DVE reciprocal + two base-aligned
   DVE multiplies normalize O^T.
 - Output projection contracts head pairs (K=128) accumulating over 6
   pairs with the two 384-wide halves' chains interleaved (weight loads
   hide under matmuls).  Bias b_eff = proj_bias + proj_weight @ v_bias
   (v_bias commutes through softmax-normalized attention) is added
   during the PSUM->SBUF move; q_bias*scale is folded into the q
   PSUM->SBUF copy (per-partition ACT bias); scale folded into wq on
   host.
"""

import numpy as np
import ml_dtypes

import concourse.bass as bass
import concourse.mybir as mybir
from concourse.bacc import Bacc
from concourse.bass_utils import run_bass_kernel_spmd
from concourse.tile import TileContext

F32 = mybir.dt.float32
BF16 = mybir.dt.bfloat16
AF = mybir.ActivationFunctionType
ALU = mybir.AluOpType

N_CORES = 8
B, NTOK, DIM = 64, 196, 768
H, HD = 12, 64
NHP = H // 2          # head pairs
BPC = B // N_CORES    # batches per core
TPC = BPC * NTOK      # tokens per core (1568)
SCALE = HD ** -0.5
KC = DIM // 128       # contraction chunks for 768 (6)
TOKC = [(0, 128), (128, 68)]   # token chunking of 196
NQCH = 4              # token N-chunks (1568/392); 392 = 2 batches
NQW = TPC // NQCH     # 392

MERGED_S_CHAIN = False  # two-MM single-bank chain crashes TRN2 (exec unit dies)
RPB_ON_GPSIMD = True    # pt = exp(S)*exp(rpb) on GpSimd vs DVE psum add


def build_nc():
    nc = Bacc()

    # x_t2[p, n, kc, w] = x_fm[kc*128+p, n*392+w] -> per-partition contiguous
    # 6*784B runs per n-chunk (big DMA packets)
    x_t = nc.declare_dram_parameter("x_t", [128, NQCH, KC, NQW], BF16, False)
    # w_all[p, w, kc, o] = W_w^T[kc*128+p, o] for w in (q,k,v,p)
    w_all = nc.declare_dram_parameter("w_all", [128, 4, KC, DIM], BF16, False)
    qb = nc.declare_dram_parameter("qb", [128, KC], F32, False)
    beff = nc.declare_dram_parameter("beff", [128, DIM], F32, False)
    # exp(rpb^T), bf16, split by ktok chunk: [j, head, i]
    rpb0 = nc.declare_dram_parameter("rpb0", [128, H, NTOK], BF16, False)
    rpb1 = nc.declare_dram_parameter("rpb1", [68, H, NTOK], BF16, False)
    y = nc.declare_dram_parameter("y", [TPC, DIM], BF16, True)

    with TileContext(nc) as tc, \
         tc.tile_pool(name="const", bufs=1) as cpool:
        def ctile(shape, dtype, nm):
            return cpool.tile(shape, dtype, name=nm, tag=nm)

        # ---------------- inputs (DMA ordered by first use) ----------------
        x_n = [ctile([128, KC, NQW], BF16, f"xn{n}") for n in range(NQCH)]
        w_t = [ctile([128, KC, DIM], BF16, f"wt{w}") for w in range(4)]
        x_sb = {(kc, n): x_n[n][:, kc, :]
                for kc in range(KC) for n in range(NQCH)}
        wq_sb = [w_t[0][:, kc, :] for kc in range(KC)]
        wk_sb = [w_t[1][:, kc, :] for kc in range(KC)]
        wv_sb = [w_t[2][:, kc, :] for kc in range(KC)]
        wp_sb = [w_t[3][:, kc, :] for kc in range(KC)]

        nc.sync.dma_start(x_n[0][:], x_t[:, 0])
        nc.sync.dma_start(w_t[0][:], w_all[:, 0])
        nc.sync.dma_start(w_t[1][:], w_all[:, 1])
        qb_sb = ctile([128, KC], F32, "qb_sb")
        nc.sync.dma_start(qb_sb[:], qb[:])
        rpb0_sb = ctile([128, H, NTOK], BF16, "rpb0_sb")
        nc.sync.dma_start(rpb0_sb[:], rpb0[:])
        rpb1_sb = ctile([68, H, NTOK], BF16, "rpb1_sb")
        nc.sync.dma_start(rpb1_sb[:], rpb1[:])
        nc.sync.dma_start(w_t[2][:], w_all[:, 2])
        for n in range(1, NQCH):
            nc.sync.dma_start(x_n[n][:], x_t[:, n])
        nc.sync.dma_start(w_t[3][:], w_all[:, 3])
        beff_bc = ctile([128, DIM], F32, "beff_bc")
        nc.sync.dma_start(beff_bc[:], beff[:])

        ones_sb = ctile([128, 128], BF16, "ones_sb")
        nc.vector.memset(ones_sb[:], 1.0)

        # ---------------- persistent activations ----------------
        q_sb = {(t, n): ctile([128, NQW], BF16, f"q{t}_{n}")
                for t in range(KC) for n in range(NQCH)}
        k_sb = {(t, n): ctile([128, NQW], BF16, f"k{t}_{n}")
                for t in range(KC) for n in range(NQCH)}
        v_sb = {}
        for b in range(BPC):
            for ci, (toff, tlen) in enumerate(TOKC):
                v_sb[(b, ci)] = ctile([tlen, NHP, 128], BF16, f"v{b}_{ci}")

        # ---------------- phase A: QKV projection ----------------
        with tc.tile_pool(name="qkv_ps", bufs=4, space="PSUM") as pqk:
            for n in range(NQCH):
                for t in range(KC):
                    psq = pqk.tile([128, NQW], F32, tag="qkps")
                    for kc in range(KC):
                        nc.tensor.matmul(
                            psq[:], wq_sb[kc][:, t * 128:(t + 1) * 128],
                            x_sb[(kc, n)][:], start=(kc == 0),
                            stop=(kc == KC - 1))
                    nc.scalar.activation(q_sb[(t, n)][:], psq[:],
                                         AF.Identity, bias=qb_sb[:, t:t + 1])
                    psk = pqk.tile([128, NQW], F32, tag="qkps")
                    for kc in range(KC):
                        nc.tensor.matmul(
                            psk[:], wk_sb[kc][:, t * 128:(t + 1) * 128],
                            x_sb[(kc, n)][:], start=(kc == 0),
                            stop=(kc == KC - 1))
                    nc.scalar.activation(k_sb[(t, n)][:], psk[:], AF.Copy)
                for b in (2 * n, 2 * n + 1):
                    for ci, (toff, tlen) in enumerate(TOKC):
                        c0 = (b % 2) * NTOK + toff
                        for nh in range(2):
                            psv = pqk.tile([128, 384], F32, tag="vps")
                            for kc in range(KC):
                                nc.tensor.matmul(
                                    psv[:tlen], x_sb[(kc, n)][:, c0:c0 + tlen],
                                    wv_sb[kc][:, nh * 384:(nh + 1) * 384],
                                    start=(kc == 0), stop=(kc == KC - 1))
                            nc.scalar.activation(
                                v_sb[(b, ci)][:, nh * 3:(nh + 1) * 3, :]
                                .rearrange("p a b -> p (a b)"),
                                psv[:tlen], AF.Copy)

        # ---------------- phase B: attention + out projection ----------------
        _ob = 2 if MERGED_S_CHAIN else 1
        with tc.tile_pool(name="s_ps", bufs=1, space="PSUM") as ps_s, \
             tc.tile_pool(name="o_ps", bufs=1, space="PSUM") as ps_o, \
             tc.tile_pool(name="r_ps", bufs=1, space="PSUM") as ps_r, \
             tc.tile_pool(name="proj_ps", bufs=2, space="PSUM") as ps_proj, \
             tc.tile_pool(name="pr_sbuf", bufs=4) as praw_pool, \
             tc.tile_pool(name="p_sbuf", bufs=4) as p_pool, \
             tc.tile_pool(name="r_sbuf", bufs=3) as r_pool, \
             tc.tile_pool(name="o_sbuf", bufs=14) as o_pool, \
             tc.tile_pool(name="y_sbuf", bufs=6) as y_pool:
            o_tiles = {}

            def stage1(b, hp):
                """S^T matmuls + exp + rpbE multiply -> p_tiles dict."""
                n = b // 2
                q0 = (b % 2) * NTOK
                p_tiles = {}
                for ci, (joff, jlen) in enumerate(TOKC):
                    rpb_sb = rpb0_sb if ci == 0 else rpb1_sb
                    jsl = slice(q0 + joff, q0 + joff + jlen)
                    rpb_pair = rpb_sb[:jlen, 2 * hp:2 * hp + 2, :] \
                        .rearrange("p h n -> p (h n)")
                    pt = p_pool.tile([jlen, 2 * NTOK], BF16, tag=f"p{ci}")
                    praw = praw_pool.tile([jlen, 2 * NTOK], BF16,
                                          tag=f"pr{ci}")
                    # [jlen, 2, 512] = one PSUM bank per head half; a single
                    # strided ACT exp then covers both heads in one op.
                    pss = ps_s.tile([jlen, 2, 512], F32, tag=f"s{ci}")
                    for hh in range(2):
                        rows = slice(hh * 64, hh * 64 + 64)
                        nc.tensor.matmul(
                            pss[:, hh, 0:NTOK], k_sb[(hp, n)][rows, jsl],
                            q_sb[(hp, n)][rows, q0:q0 + NTOK],
                            start=True, stop=True)
                    nc.scalar.activation(
                        praw[:].rearrange("p (a b) -> p a b", a=2),
                        pss[:, :, 0:NTOK], AF.Exp)
                    # rpbE multiply split so the first PV quarter-chain can
                    # start after ~half the multiply latency: ci0/head0 on
                    # GpSimd, the rest on DVE (runs concurrently).
                    if ci == 0:
                        nc.gpsimd.tensor_tensor(pt[:, 0:NTOK],
                                                praw[:, 0:NTOK],
                                                rpb_pair[:, 0:NTOK], ALU.mult)
                        nc.vector.tensor_tensor(pt[:, NTOK:2 * NTOK],
                                                praw[:, NTOK:2 * NTOK],
                                                rpb_pair[:, NTOK:2 * NTOK],
                                                ALU.mult)
                    else:
                        nc.vector.tensor_tensor(pt[:], praw[:], rpb_pair,
                                                ALU.mult)
                    p_tiles[ci] = pt
                return p_tiles

            def stage2(b, hp, p_tiles):
                """PV + sum matmuls, reciprocal, normalize -> o_tiles.

                One [128, 392] matmul per ktok chunk computes BOTH heads'
                O^T: lhsT = [v_h0 | v_h1] (128 cols), rhs = [P_h0^T P_h1^T]
                (392 cols).  Diagonal 64x196 blocks are the real outputs;
                off-diagonal blocks are discarded.  Same streamed columns
                as two separate matmuls, half the weight loads.  The sums
                come from one ones[jlen,128] matmul the same way.
                """
                po = ps_o.tile([128, 2 * NTOK], F32, tag="o")
                psr = ps_r.tile([128, NTOK], F32, tag="r")
                # PV as a 4-MM chain (per ci x head-half of the streamed P)
                # so the first MM only waits on the ci0/head0 multiply.
                for ci, (joff, jlen) in enumerate(TOKC):
                    for hh in range(2):
                        nc.tensor.matmul(
                            po[:, hh * NTOK:(hh + 1) * NTOK],
                            v_sb[(b, ci)][:, hp, :],
                            p_tiles[ci][:, hh * NTOK:(hh + 1) * NTOK],
                            start=(ci == 0 and hh == 0),
                            stop=(ci == 1 and hh == 1))
                for hh in range(2):
                    cols = slice(hh * 64, hh * 64 + 64)
                    for ci, (joff, jlen) in enumerate(TOKC):
                        nc.tensor.matmul(
                            psr[cols, :], ones_sb[:jlen, 0:64],
                            p_tiles[ci][:, hh * NTOK:(hh + 1) * NTOK],
                            start=(ci == 0), stop=(ci == 1))
                rbc = r_pool.tile([128, NTOK], F32, tag="rbc")
                nc.vector.reciprocal_approx_fast(out=rbc[:], in_=psr[:])
                ot = o_pool.tile([128, NTOK], BF16, tag="o_sb")
                nc.vector.tensor_tensor(ot[0:64, :], po[0:64, 0:NTOK],
                                        rbc[0:64, :], ALU.mult)
                nc.vector.tensor_tensor(ot[64:128, :],
                                        po[64:128, NTOK:2 * NTOK],
                                        rbc[64:128, :], ALU.mult)
                o_tiles[(b, hp)] = ot

            def proj(b):
                for ci, (toff, tlen) in enumerate(TOKC):
                    psy = [ps_proj.tile([128, 384], F32, tag="proj",
                                        name=f"psy{nh}") for nh in range(2)]
                    for hp in range(NHP):
                        for nh in range(2):
                            nc.tensor.matmul(
                                psy[nh][:tlen],
                                o_tiles[(b, hp)][:, toff:toff + tlen],
                                wp_sb[hp][:, nh * 384:(nh + 1) * 384],
                                start=(hp == 0), stop=(hp == NHP - 1))
                    tok0 = b * NTOK + toff
                    for nh in range(2):
                        yt = y_pool.tile([128, 384], BF16, tag="y")
                        nc.vector.tensor_tensor(
                            yt[:tlen], psy[nh][:tlen],
                            beff_bc[:tlen, nh * 384:(nh + 1) * 384], ALU.add)
                        nc.sync.dma_start(
                            y[tok0:tok0 + tlen, nh * 384:(nh + 1) * 384],
                            yt[:tlen])

            # Software-pipelined emission: the S-stage runs STAGGER blocks
            # ahead of the PV-stage so the in-order PE queue never stalls
            # on the exp -> rpbE-multiply chain.
            STAGGER = 0
            blocks = [(b, hp) for b in range(BPC) for hp in range(NHP)]
            pending = {}
            for idx in range(min(STAGGER, len(blocks))):
                pending[idx] = stage1(*blocks[idx])
            for k, (b, hp) in enumerate(blocks):
                nxt = k + STAGGER
                if nxt < len(blocks):
                    pending[nxt] = stage1(*blocks[nxt])
                stage2(b, hp, pending.pop(k))
                if hp == NHP - 1:
                    proj(b)
    nc.finalize()
    return nc


def prep_host(x, qkv_weight, q_bias, v_bias, rpb_table, rel_pos_index,
              proj_weight, proj_bias):
    """Host-side prep: transposes, dtype casts, bias folding, rpb gather."""
    bf16 = ml_dtypes.bfloat16
    x = np.asarray(x, np.float32)
    qkv_weight = np.asarray(qkv_weight, np.float32)
    proj_weight = np.asarray(proj_weight, np.float32)
    q_bias = np.asarray(q_bias, np.float32)
    v_bias = np.asarray(v_bias, np.float32)
    rpb_table = np.asarray(rpb_table, np.float32)
    rel_pos_index = np.asarray(rel_pos_index)
    proj_bias = np.asarray(proj_bias, np.float32)

    # scale folded into q projection weights + bias
    wq = qkv_weight[0:DIM].T * SCALE
    wk = qkv_weight[DIM:2 * DIM].T
    wv = qkv_weight[2 * DIM:3 * DIM].T
    wp = proj_weight.T
    # w_all[p, w, kc, o] = W_w^T[kc*128+p, o]
    w_all = np.stack([w.reshape(KC, 128, DIM) for w in (wq, wk, wv, wp)],
                     axis=0)                     # [4, kc, p, o]
    w_all = np.ascontiguousarray(
        w_all.transpose(2, 0, 1, 3)).astype(bf16)  # [p, 4, kc, o]
    qb = np.ascontiguousarray((q_bias * SCALE).reshape(KC, 128).T).astype(np.float32)
    beff = np.ascontiguousarray(np.broadcast_to(
        (proj_bias + proj_weight @ v_bias).reshape(1, DIM), (128, DIM))).astype(np.float32)

    rpb_full = rpb_table[rel_pos_index.reshape(-1)].reshape(NTOK, NTOK, H)
    rpbT = np.exp(np.ascontiguousarray(rpb_full.transpose(1, 2, 0)),
                  dtype=np.float32)
    rpb0 = np.ascontiguousarray(rpbT[0:128]).astype(bf16)
    rpb1 = np.ascontiguousarray(rpbT[128:NTOK]).astype(bf16)

    shared = dict(w_all=w_all, qb=qb, beff=beff, rpb0=rpb0, rpb1=rpb1)
    in_maps = []
    for c in range(N_CORES):
        xs = x[c * BPC:(c + 1) * BPC]                       # [8,196,768]
        x_fm = xs.transpose(2, 0, 1).reshape(DIM, TPC)      # [768, 1568]
        # x_t2[p, n, kc, w] = x_fm[kc*128+p, n*392+w]
        x_tc = np.ascontiguousarray(
            x_fm.reshape(KC, 128, NQCH, NQW).transpose(1, 2, 0, 3)
        ).astype(bf16)
        in_maps.append(dict(shared, x_t=x_tc))
    return in_maps


_NC_CACHE = {}


def get_nc():
    if "nc" not in _NC_CACHE:
        _NC_CACHE["nc"] = build_nc()
    return _NC_CACHE["nc"]


def kernel(**inputs):
    nc = get_nc()
    in_maps = prep_host(**inputs)
    res = run_bass_kernel_spmd(nc, in_maps, list(range(N_CORES)))
    outs = [res.results[c]["y"].reshape(BPC, NTOK, DIM) for c in range(N_CORES)]
    return np.concatenate(outs, axis=0).astype(np.float32)



# revision 25
# speedup vs baseline: 1.0615x; 1.0615x over previous
"""Trainium2 Bass kernel for Swin-style windowed attention.

Problem: x[64,196,768] -> qkv proj -> 12-head attention with relative
position bias -> out proj.  Sharded data-parallel over batch: 8 batch
items per NeuronCore across 8 cores.  All matmuls bf16 with fp32 PSUM
accumulation (measured rel err ~4e-3 vs the fp32 reference).

Per-core design (8 batch items, ~210 us on HW):
 - QKV projection: q,k feature-major ([feat, tok] so each head's 64-dim
   slice sits on partitions), v token-major ([tok, head-pair, 128]).
   x/q/k are tiled per 392-token chunk; inputs arrive via per-partition-
   contiguous repacked layouts (big DMA packets), ordered by first use.
 - Attention in the S^T layout, head pairs processed together:
   S^T[j,i] = sum_d k[d,j] q[d,i] (K=64, the pair row-packed into the
   128-deep PE array).  The pair's two S^T tiles live in ONE two-bank
   psum tile [jlen, 2, 512] (one bank per head; two accumulation chains
   in a single bank crash the exec unit), so a single strided ACT exp
   covers both heads.  Softmax runs along partitions (no max-subtract:
   logits are O(1) by construction).
 - Relative position bias applied multiplicatively after exp:
   pt = exp(S^T) * exp(rpb^T), with exp(rpb^T) precomputed on host; the
   multiply runs on the otherwise-idle GpSimd engine for the 128-row
   chunk and on DVE for the 68-row chunk.
 - PV: ONE [128, 2*196] matmul per ktok chunk with lhsT = [v_h0|v_h1]
   and rhs = [P_h0^T P_h1^T]: the diagonal 64x196 blocks are both
   heads' O^T, off-diagonal blocks are discarded -- same streamed
   columns as two matmuls, half the weight loads.  Softmax sums come
   from ones[jlen,64] matmuls (replicating each head's sums across its
   64 rows, col-packed); one fast DVE reciprocal + two base-aligned
   DVE multiplies normalize O^T.
 - Output projection contracts head pairs (K=128) accumulating over 6
   pairs with the two 384-wide halves' chains interleaved (weight loads
   hide under matmuls).  Bias b_eff = proj_bias + proj_weight @ v_bias
   (v_bias commutes through softmax-normalized attention) is added
   during the PSUM->SBUF move; q_bias*scale is folded into the q
   PSUM->SBUF copy (per-partition ACT bias); scale folded into wq on
   host.
"""

import numpy as np
import ml_dtypes

import concourse.bass as bass
import concourse.mybir as mybir
from concourse.bacc import Bacc
from concourse.bass_utils import run_bass_kernel_spmd
from concourse.tile import TileContext

F32 = mybir.dt.float32
BF16 = mybir.dt.bfloat16
AF = mybir.ActivationFunctionType
ALU = mybir.AluOpType

N_CORES = 8
B, NTOK, DIM = 64, 196, 768
H, HD = 12, 64
NHP = H // 2          # head pairs
BPC = B // N_CORES    # batches per core
TPC = BPC * NTOK      # tokens per core (1568)
SCALE = HD ** -0.5
KC = DIM // 128       # contraction chunks for 768 (6)
TOKC = [(0, 128), (128, 68)]   # token chunking of 196
NQCH = 4              # token N-chunks (1568/392); 392 = 2 batches
NQW = TPC // NQCH     # 392

MERGED_S_CHAIN = False  # two-MM single-bank chain crashes TRN2 (exec unit dies)
RPB_ON_GPSIMD = True    # pt = exp(S)*exp(rpb) on GpSimd vs DVE psum add


def build_nc():
    nc = Bacc()

    # x_t2[p, n, kc, w] = x_fm[kc*128+p, n*392+w] -> per-partition contiguous
    # 6*784B runs per n-chunk (big DMA packets)
    x_t = nc.declare_dram_parameter("x_t", [128, NQCH, KC, NQW], BF16, False)
    # w_all[p, w, kc, o] = W_w^T[kc*128+p, o] for w in (q,k,v,p)
    w_all = nc.declare_dram_parameter("w_all", [128, 4, KC, DIM], BF16, False)
    qb = nc.declare_dram_parameter("qb", [128, KC], F32, False)
    beff = nc.declare_dram_parameter("beff", [128, DIM], F32, False)
    # exp(rpb^T), bf16, split by ktok chunk: [j, head, i]
    rpb0 = nc.declare_dram_parameter("rpb0", [128, H, NTOK], BF16, False)
    rpb1 = nc.declare_dram_parameter("rpb1", [68, H, NTOK], BF16, False)
    y = nc.declare_dram_parameter("y", [TPC, DIM], BF16, True)

    with TileContext(nc) as tc, \
         tc.tile_pool(name="const", bufs=1) as cpool:
        def ctile(shape, dtype, nm):
            return cpool.tile(shape, dtype, name=nm, tag=nm)

        # ---------------- inputs (DMA ordered by first use) ----------------
        x_n = [ctile([128, KC, NQW], BF16, f"xn{n}") for n in range(NQCH)]
        w_t = [ctile([128, KC, DIM], BF16, f"wt{w}") for w in range(4)]
        x_sb = {(kc, n): x_n[n][:, kc, :]
                for kc in range(KC) for n in range(NQCH)}
        wq_sb = [w_t[0][:, kc, :] for kc in range(KC)]
        wk_sb = [w_t[1][:, kc, :] for kc in range(KC)]
        wv_sb = [w_t[2][:, kc, :] for kc in range(KC)]
        wp_sb = [w_t[3][:, kc, :] for kc in range(KC)]

        nc.sync.dma_start(x_n[0][:], x_t[:, 0])
        nc.sync.dma_start(w_t[0][:], w_all[:, 0])
        nc.sync.dma_start(w_t[1][:], w_all[:, 1])
        qb_sb = ctile([128, KC], F32, "qb_sb")
        nc.sync.dma_start(qb_sb[:], qb[:])
        rpb0_sb = ctile([128, H, NTOK], BF16, "rpb0_sb")
        nc.sync.dma_start(rpb0_sb[:], rpb0[:])
        rpb1_sb = ctile([68, H, NTOK], BF16, "rpb1_sb")
        nc.sync.dma_start(rpb1_sb[:], rpb1[:])
        nc.sync.dma_start(w_t[2][:], w_all[:, 2])
        for n in range(1, NQCH):
            nc.sync.dma_start(x_n[n][:], x_t[:, n])
        nc.sync.dma_start(w_t[3][:], w_all[:, 3])
        beff_bc = ctile([128, DIM], F32, "beff_bc")
        nc.sync.dma_start(beff_bc[:], beff[:])

        ones_sb = ctile([128, 128], BF16, "ones_sb")
        nc.vector.memset(ones_sb[:], 1.0)

        # PE warm-up: dummy matmuls with no DMA deps run while the input
        # DMAs land (first real MM is ~14.5us in), so the HAM clock gate is
        # already at 2.4 GHz when the QKV matmuls start.
        with tc.tile_pool(name="warm_ps", bufs=1, space="PSUM") as ps_w:
            wps = ps_w.tile([128, 128], F32, tag="w")
            for _ in range(96):
                nc.tensor.matmul(wps[:], ones_sb[:], ones_sb[:],
                                 start=True, stop=True)

        # ---------------- persistent activations ----------------
        q_sb = {(t, n): ctile([128, NQW], BF16, f"q{t}_{n}")
                for t in range(KC) for n in range(NQCH)}
        k_sb = {(t, n): ctile([128, NQW], BF16, f"k{t}_{n}")
                for t in range(KC) for n in range(NQCH)}
        v_sb = {}
        for b in range(BPC):
            for ci, (toff, tlen) in enumerate(TOKC):
                v_sb[(b, ci)] = ctile([tlen, NHP, 128], BF16, f"v{b}_{ci}")

        # ---------------- phase A: QKV projection ----------------
        with tc.tile_pool(name="qkv_ps", bufs=4, space="PSUM") as pqk:
            for n in range(NQCH):
                for t in range(KC):
                    psq = pqk.tile([128, NQW], F32, tag="qkps")
                    for kc in range(KC):
                        nc.tensor.matmul(
                            psq[:], wq_sb[kc][:, t * 128:(t + 1) * 128],
                            x_sb[(kc, n)][:], start=(kc == 0),
                            stop=(kc == KC - 1))
                    nc.scalar.activation(q_sb[(t, n)][:], psq[:],
                                         AF.Identity, bias=qb_sb[:, t:t + 1])
                    psk = pqk.tile([128, NQW], F32, tag="qkps")
                    for kc in range(KC):
                        nc.tensor.matmul(
                            psk[:], wk_sb[kc][:, t * 128:(t + 1) * 128],
                            x_sb[(kc, n)][:], start=(kc == 0),
                            stop=(kc == KC - 1))
                    nc.scalar.activation(k_sb[(t, n)][:], psk[:], AF.Copy)
                for b in (2 * n, 2 * n + 1):
                    for ci, (toff, tlen) in enumerate(TOKC):
                        c0 = (b % 2) * NTOK + toff
                        for nh in range(2):
                            psv = pqk.tile([128, 384], F32, tag="vps")
                            for kc in range(KC):
                                nc.tensor.matmul(
                                    psv[:tlen], x_sb[(kc, n)][:, c0:c0 + tlen],
                                    wv_sb[kc][:, nh * 384:(nh + 1) * 384],
                                    start=(kc == 0), stop=(kc == KC - 1))
                            nc.scalar.activation(
                                v_sb[(b, ci)][:, nh * 3:(nh + 1) * 3, :]
                                .rearrange("p a b -> p (a b)"),
                                psv[:tlen], AF.Copy)

        # ---------------- phase B: attention + out projection ----------------
        _ob = 2 if MERGED_S_CHAIN else 1
        with tc.tile_pool(name="s_ps", bufs=1, space="PSUM") as ps_s, \
             tc.tile_pool(name="o_ps", bufs=1, space="PSUM") as ps_o, \
             tc.tile_pool(name="r_ps", bufs=1, space="PSUM") as ps_r, \
             tc.tile_pool(name="proj_ps", bufs=2, space="PSUM") as ps_proj, \
             tc.tile_pool(name="pr_sbuf", bufs=4) as praw_pool, \
             tc.tile_pool(name="p_sbuf", bufs=4) as p_pool, \
             tc.tile_pool(name="r_sbuf", bufs=3) as r_pool, \
             tc.tile_pool(name="o_sbuf", bufs=14) as o_pool, \
             tc.tile_pool(name="y_sbuf", bufs=6) as y_pool:
            o_tiles = {}

            def stage1(b, hp):
                """S^T matmuls + exp + rpbE multiply -> p_tiles dict."""
                n = b // 2
                q0 = (b % 2) * NTOK
                p_tiles = {}
                for ci, (joff, jlen) in enumerate(TOKC):
                    rpb_sb = rpb0_sb if ci == 0 else rpb1_sb
                    jsl = slice(q0 + joff, q0 + joff + jlen)
                    rpb_pair = rpb_sb[:jlen, 2 * hp:2 * hp + 2, :] \
                        .rearrange("p h n -> p (h n)")
                    pt = p_pool.tile([jlen, 2 * NTOK], BF16, tag=f"p{ci}")
                    praw = praw_pool.tile([jlen, 2 * NTOK], BF16,
                                          tag=f"pr{ci}")
                    # [jlen, 2, 512] = one PSUM bank per head half; a single
                    # strided ACT exp then covers both heads in one op.
                    pss = ps_s.tile([jlen, 2, 512], F32, tag=f"s{ci}")
                    for hh in range(2):
                        rows = slice(hh * 64, hh * 64 + 64)
                        nc.tensor.matmul(
                            pss[:, hh, 0:NTOK], k_sb[(hp, n)][rows, jsl],
                            q_sb[(hp, n)][rows, q0:q0 + NTOK],
                            start=True, stop=True)
                    nc.scalar.activation(
                        praw[:].rearrange("p (a b) -> p a b", a=2),
                        pss[:, :, 0:NTOK], AF.Exp)
                    eng = nc.gpsimd if ci == 0 else nc.vector
                    eng.tensor_tensor(pt[:], praw[:], rpb_pair, ALU.mult)
                    p_tiles[ci] = pt
                return p_tiles

            def stage2(b, hp, p_tiles):
                """PV + sum matmuls, reciprocal, normalize -> o_tiles.

                One [128, 392] matmul per ktok chunk computes BOTH heads'
                O^T: lhsT = [v_h0 | v_h1] (128 cols), rhs = [P_h0^T P_h1^T]
                (392 cols).  Diagonal 64x196 blocks are the real outputs;
                off-diagonal blocks are discarded.  Same streamed columns
                as two separate matmuls, half the weight loads.  The sums
                come from one ones[jlen,128] matmul the same way.
                """
                po = ps_o.tile([128, 2 * NTOK], F32, tag="o")
                psr = ps_r.tile([128, NTOK], F32, tag="r")
                for ci, (joff, jlen) in enumerate(TOKC):
                    nc.tensor.matmul(po[:], v_sb[(b, ci)][:, hp, :],
                                     p_tiles[ci][:], start=(ci == 0),
                                     stop=(ci == 1))
                for hh in range(2):
                    cols = slice(hh * 64, hh * 64 + 64)
                    for ci, (joff, jlen) in enumerate(TOKC):
                        nc.tensor.matmul(
                            psr[cols, :], ones_sb[:jlen, 0:64],
                            p_tiles[ci][:, hh * NTOK:(hh + 1) * NTOK],
                            start=(ci == 0), stop=(ci == 1))
                rbc = r_pool.tile([128, NTOK], F32, tag="rbc")
                nc.vector.reciprocal_approx_fast(out=rbc[:], in_=psr[:])
                ot = o_pool.tile([128, NTOK], BF16, tag="o_sb")
                nc.vector.tensor_tensor(ot[0:64, :], po[0:64, 0:NTOK],
                                        rbc[0:64, :], ALU.mult)
                nc.vector.tensor_tensor(ot[64:128, :],
                                        po[64:128, NTOK:2 * NTOK],
                                        rbc[64:128, :], ALU.mult)
                o_tiles[(b, hp)] = ot

            def proj(b):
                for ci, (toff, tlen) in enumerate(TOKC):
                    psy = [ps_proj.tile([128, 384], F32, tag="proj",
                                        name=f"psy{nh}") for nh in range(2)]
                    for hp in range(NHP):
                        for nh in range(2):
                            nc.tensor.matmul(
                                psy[nh][:tlen],
                                o_tiles[(b, hp)][:, toff:toff + tlen],
                                wp_sb[hp][:, nh * 384:(nh + 1) * 384],
                                start=(hp == 0), stop=(hp == NHP - 1))
                    tok0 = b * NTOK + toff
                    for nh in range(2):
                        yt = y_pool.tile([128, 384], BF16, tag="y")
                        nc.vector.tensor_tensor(
                            yt[:tlen], psy[nh][:tlen],
                            beff_bc[:tlen, nh * 384:(nh + 1) * 384], ALU.add)
                        nc.sync.dma_start(
                            y[tok0:tok0 + tlen, nh * 384:(nh + 1) * 384],
                            yt[:tlen])

            # Software-pipelined emission: the S-stage runs STAGGER blocks
            # ahead of the PV-stage so the in-order PE queue never stalls
            # on the exp -> rpbE-multiply chain.
            STAGGER = 0
            blocks = [(b, hp) for b in range(BPC) for hp in range(NHP)]
            pending = {}
            for idx in range(min(STAGGER, len(blocks))):
                pending[idx] = stage1(*blocks[idx])
            for k, (b, hp) in enumerate(blocks):
                nxt = k + STAGGER
                if nxt < len(blocks):
                    pending[nxt] = stage1(*blocks[nxt])
                stage2(b, hp, pending.pop(k))
                if hp == NHP - 1:
                    proj(b)
    nc.finalize()
    return nc


def prep_host(x, qkv_weight, q_bias, v_bias, rpb_table, rel_pos_index,
              proj_weight, proj_bias):
    """Host-side prep: transposes, dtype casts, bias folding, rpb gather."""
    bf16 = ml_dtypes.bfloat16
    x = np.asarray(x, np.float32)
    qkv_weight = np.asarray(qkv_weight, np.float32)
    proj_weight = np.asarray(proj_weight, np.float32)
    q_bias = np.asarray(q_bias, np.float32)
    v_bias = np.asarray(v_bias, np.float32)
    rpb_table = np.asarray(rpb_table, np.float32)
    rel_pos_index = np.asarray(rel_pos_index)
    proj_bias = np.asarray(proj_bias, np.float32)

    # scale folded into q projection weights + bias
    wq = qkv_weight[0:DIM].T * SCALE
    wk = qkv_weight[DIM:2 * DIM].T
    wv = qkv_weight[2 * DIM:3 * DIM].T
    wp = proj_weight.T
    # w_all[p, w, kc, o] = W_w^T[kc*128+p, o]
    w_all = np.stack([w.reshape(KC, 128, DIM) for w in (wq, wk, wv, wp)],
                     axis=0)                     # [4, kc, p, o]
    w_all = np.ascontiguousarray(
        w_all.transpose(2, 0, 1, 3)).astype(bf16)  # [p, 4, kc, o]
    qb = np.ascontiguousarray((q_bias * SCALE).reshape(KC, 128).T).astype(np.float32)
    beff = np.ascontiguousarray(np.broadcast_to(
        (proj_bias + proj_weight @ v_bias).reshape(1, DIM), (128, DIM))).astype(np.float32)

    rpb_full = rpb_table[rel_pos_index.reshape(-1)].reshape(NTOK, NTOK, H)
    rpbT = np.exp(np.ascontiguousarray(rpb_full.transpose(1, 2, 0)),
                  dtype=np.float32)
    rpb0 = np.ascontiguousarray(rpbT[0:128]).astype(bf16)
    rpb1 = np.ascontiguousarray(rpbT[128:NTOK]).astype(bf16)

    shared = dict(w_all=w_all, qb=qb, beff=beff, rpb0=rpb0, rpb1=rpb1)
    in_maps = []
    for c in range(N_CORES):
        xs = x[c * BPC:(c + 1) * BPC]                       # [8,196,768]
        x_fm = xs.transpose(2, 0, 1).reshape(DIM, TPC)      # [768, 1568]
        # x_t2[p, n, kc, w] = x_fm[kc*128+p, n*392+w]
        x_tc = np.ascontiguousarray(
            x_fm.reshape(KC, 128, NQCH, NQW).transpose(1, 2, 0, 3)
        ).astype(bf16)
        in_maps.append(dict(shared, x_t=x_tc))
    return in_maps


_NC_CACHE = {}


def get_nc():
    if "nc" not in _NC_CACHE:
        _NC_CACHE["nc"] = build_nc()
    return _NC_CACHE["nc"]


def kernel(**inputs):
    nc = get_nc()
    in_maps = prep_host(**inputs)
    res = run_bass_kernel_spmd(nc, in_maps, list(range(N_CORES)))
    outs = [res.results[c]["y"].reshape(BPC, NTOK, DIM) for c in range(N_CORES)]
    return np.concatenate(outs, axis=0).astype(np.float32)

